# revision 1
# baseline (speedup 1.0000x reference)
"""Trainium2 Bass kernel for nn_Attention_Net (encoder GRU + Bahdanau-style
attention + decoder GRU + output head).

Key algebraic simplification: the attention score is
    e[b, l] = (s @ wa_s)[b] + h_proj[b, l] + ba
i.e. a per-batch scalar plus a step-independent vector. Softmax is
shift-invariant, so alpha = softmax(h_proj) is CONSTANT across decoder steps.
The context c and the decoder input gates gi_d are therefore computed once and
the decoder collapses to a plain GRU recurrence with constant input.

Sharding: data-parallel over batch B=64 across 8 cores (8 batch each),
weights replicated. No collectives.

Layout: hidden dim on partitions, batch on the free dim. The recurrent
matmul per step is gh.T[j, b] = sum_k W_hh[j, k] h[k, b], done as 12
[128x128]x[128x8] fp16 matmuls (weights stationary). Gates are fp32
elementwise on [128, gate, 8] tiles; state is carried fp16.
"""

import sys
import numpy as np

for _p in ("/opt/trn_rl_repo", "/root/.axon_site/_ro/trn_rl_repo"):
    if _p not in sys.path:
        sys.path.append(_p)

import concourse.bass as bass
import concourse.tile as tile
from concourse import bacc, mybir
from concourse.bass_utils import run_bass_kernel_spmd

F32 = mybir.dt.float32
F16 = mybir.dt.float16

B, L, P, H, OUT = 64, 1024, 64, 256, 128
NCORES = 8
BS = B // NCORES          # 8 batch per core
BODY = 128                # steps per For_i iteration
NB = L // BODY            # 8 loop iterations per scan
AF = mybir.ActivationFunctionType
ALU = mybir.AluOpType


def build_program(nb=NB):
    """Emit the SPMD single-core program. nb = number of 128-step loop
    iterations per scan (nb=NB for the real kernel; smaller for sim tests)."""
    Ls = nb * BODY                     # sequence length this build handles
    nc = bacc.Bacc()

    # ---- DRAM I/O (per-core values supplied via in_maps) ----
    xT = nc.dram_tensor("xT", [P, nb * 1024 + 1024], F16, kind="ExternalInput")
    wenc = nc.dram_tensor("wenc", [128, 1536], F16, kind="ExternalInput")
    wdec = nc.dram_tensor("wdec", [128, 1536], F16, kind="ExternalInput")
    wihd = nc.dram_tensor("wihd", [128, 1536], F16, kind="ExternalInput")
    wihe = nc.dram_tensor("wihe", [P, 768], F16, kind="ExternalInput")
    gibias_e = nc.dram_tensor("gibias_e", [128, 6], F32, kind="ExternalInput")
    gidbias = nc.dram_tensor("gidbias", [128, 6, BS], F32, kind="ExternalInput")
    bhhn_e = nc.dram_tensor("bhhn_e", [128, 2, BS], F32, kind="ExternalInput")
    bhhn_d = nc.dram_tensor("bhhn_d", [128, 2, BS], F32, kind="ExternalInput")
    wah_rep = nc.dram_tensor("wah_rep", [128, 256], F16, kind="ExternalInput")
    wdo1 = nc.dram_tensor("wdo1", [128, 2], F16, kind="ExternalInput")
    bdo = nc.dram_tensor("bdo", [128, 1], F32, kind="ExternalInput")
    bmask = nc.dram_tensor("bmask", [128, BS], F16, kind="ExternalInput")
    ident8 = nc.dram_tensor("ident8", [BS, BS], F16, kind="ExternalInput")
    gidbrow = nc.dram_tensor("gidbrow", [1, 512], F16, kind="ExternalInput")
    wout = nc.dram_tensor("wout", [128, nb * 1024], F16, kind="ExternalInput")
    bout = nc.dram_tensor("bout", [128, 1], F32, kind="ExternalInput")
    out_t = nc.dram_tensor("out_t", [128, BS], F32, kind="ExternalOutput")

    with tile.TileContext(nc) as tc:
        with tc.tile_pool(name="persist", bufs=1) as persist, \
             tc.tile_pool(name="gates", bufs=3) as gates, \
             tc.tile_pool(name="xblk", bufs=2) as xblkp, \
             tc.tile_pool(name="psg", bufs=3, space="PSUM") as psg, \
             tc.tile_pool(name="psbig", bufs=2, space="PSUM") as psbig:

            # ---- persistent SBUF tiles ----
            wenc_sb = persist.tile([128, 1536], F16)
            wdec_sb = persist.tile([128, 1536], F16)
            wihd_sb = persist.tile([128, 1536], F16)
            wihe_sb = persist.tile([P, 768], F16)
            gibe_sb = persist.tile([128, 6], F32)
            gid_bias_sb = persist.tile([128, 6, BS], F32)
            bhne_sb = persist.tile([128, 2, BS], F32)
            bhnd_sb = persist.tile([128, 2, BS], F32)
            wah_sb = persist.tile([128, 256], F16)
            wdo_sb = persist.tile([128, 2], F16)
            bdo_sb = persist.tile([128, 1], F32)
            bmask_sb = persist.tile([128, BS], F16)
            id8_sb = persist.tile([BS, BS], F16)
            gidbrow_sb = persist.tile([1, 512], F16)
            gidT_sb = persist.tile([BS, 512], F16)
            wout_sb = persist.tile([128, nb * 1024], F16)
            bout_sb = persist.tile([128, 1], F32)

            h_all = persist.tile([128, (Ls + 1) * 2 * BS], F16)   # col = s*16+kh*8+b
            gi_a = persist.tile([128, 6, 64, BS], F32)            # [g, j, b]
            gi_b = persist.tile([128, 6, 64, BS], F32)
            s_all = persist.tile([128, Ls * 2 * BS], F16)         # col = i*16+kh*8+b
            h_ring = persist.tile([128, (BODY + 1) * 2 * BS], F16)
            s_ring = persist.tile([128, (BODY + 1) * 2 * BS], F16)
            y128 = persist.tile([128, nb * 8], F32)   # [(di,b) part, cc free]
            E_bc = persist.tile([128, nb * 128 * BS], F16)        # exp(h_proj) replicated
            ttr_scr = persist.tile([128, Ls], F16)
            gid_full = persist.tile([128, 6, BS], F32)
            c16 = persist.tile([128, 2, BS], F16)
            out_sb = persist.tile([128, BS], F32)

            # ---- load constants ----
            for dst, src in [(wenc_sb, wenc), (wdec_sb, wdec),
                             (wihd_sb, wihd), (wihe_sb, wihe),
                             (gibe_sb, gibias_e), (gid_bias_sb, gidbias),
                             (bhne_sb, bhhn_e), (bhnd_sb, bhhn_d),
                             (wah_sb, wah_rep), (wdo_sb, wdo1), (bdo_sb, bdo),
                             (bmask_sb, bmask), (id8_sb, ident8),
                             (gidbrow_sb, gidbrow),
                             (wout_sb, wout), (bout_sb, bout)]:
                nc.sync.dma_start(out=dst[:], in_=src[:])

            nc.vector.memset(h_all[:, 0:2 * BS], 0.0)   # h_0 = 0 (slot 0)

            # ---- helpers ----
            def load_xblk(xcol_off):
                """DMA one 64-step block of x.T into SBUF (dynamic DRAM col)."""
                xb = xblkp.tile([P, 512], F16)
                nc.sync.dma_start(out=xb[:], in_=xT[:, bass.ds(xcol_off, 512)])
                return xb

            def emit_gi_block(xb, target):
                """gi for 64 steps: target[:, g, j, b] = sum_p W_ih_e[g*128+:,p]
                * x[p, (j, b)] + bias."""
                for g in range(6):
                    ps = psbig.tile([128, 512], F32)
                    nc.tensor.matmul(
                        ps[:],
                        lhsT=wihe_sb[:, g * 128:(g + 1) * 128],
                        rhs=xb[:],
                        start=True, stop=True)
                    nc.scalar.activation(
                        target[:, g, :, :],
                        ps[:].rearrange("p (j b) -> p j b", b=BS),
                        AF.Identity, bias=gibe_sb[:, g:g + 1])

            def gru_step(w_sb, rhs_slice_fn, h_prev_ap, h_out_ap,
                         girz_ap, gin_ap, bhn_sb, gidT=None):
                """One GRU step. psum[:, g, :] = sum_k W.T_tile(k,g) @ h_prev.
                girz_ap: [128,4,BS] input gates r,z (biases folded).
                gin_ap: [128,2,BS] input gate n (b_ih_n folded)."""
                ps = psg.tile([128, 6, BS], F32, tag="ps")
                for g in (0, 1, 2, 3, 4, 5):
                    fold = gidT is not None and g < 4
                    for k in (0, 1):
                        nc.tensor.matmul(
                            ps[:, g, :],
                            lhsT=w_sb[:, (k * 6 + g) * 128:(k * 6 + g + 1) * 128],
                            rhs=rhs_slice_fn(k),
                            start=(k == 0), stop=(k == 1) and not fold)
                    if fold:
                        # constant input-gates folded into the accumulation:
                        # ps[:,g,b] += sum_q gidT[q, g*128+:] * I8[q, b]
                        nc.tensor.matmul(
                            ps[:, g, :],
                            lhsT=gidT[0:BS, g * 128:(g + 1) * 128],
                            rhs=id8_sb[:], start=False, stop=True)
                rzs = gates.tile([128, 4, BS], F32)
                if gidT is not None:
                    nc.scalar.activation(rzs[:], ps[:, 0:4, :], AF.Sigmoid)
                else:
                    rz = gates.tile([128, 4, BS], F32)
                    nc.vector.tensor_add(rz[:], ps[:, 0:4, :], girz_ap)
                    nc.scalar.activation(rzs[:], rz[:], AF.Sigmoid)
                hn = gates.tile([128, 2, BS], F32)
                nc.vector.tensor_add(hn[:], ps[:, 4:6, :], bhn_sb[:])
                rhn = gates.tile([128, 2, BS], F32)
                nc.vector.tensor_mul(rhn[:], rzs[:, 0:2, :], hn[:])
                nin = gates.tile([128, 2, BS], F32)
                nc.vector.tensor_add(nin[:], rhn[:], gin_ap)
                n_t = gates.tile([128, 2, BS], F32)
                nc.scalar.activation(n_t[:], nin[:], AF.Tanh)
                d_t = gates.tile([128, 2, BS], F32)
                nc.vector.tensor_sub(d_t[:], h_prev_ap, n_t[:])
                zd = gates.tile([128, 2, BS], F32)
                nc.vector.tensor_mul(zd[:], rzs[:, 2:4, :], d_t[:])
                nc.vector.tensor_add(h_out_ap, n_t[:], zd[:])

            # ---- encoder prologue: gi blocks 0, 1; h_ring slot 0 = 0 ----
            emit_gi_block(load_xblk(0), gi_a)
            emit_gi_block(load_xblk(512), gi_b)
            nc.vector.memset(h_ring[:, 0:2 * BS], 0.0)

            # ---- encoder scan (static ring addressing; DMA flush to h_all) --
            HE = (mybir.EngineType.PE, mybir.EngineType.DVE,
                  mybir.EngineType.Activation)
            W2 = 2 * BS
            HB = BODY // 2 * W2                     # ring cols per half-body
            with tc.For_i(0, nb, 1, hint_engines=HE,
                          staggered_reset=True) as iv:
                for j in range(BODY):
                    gi = gi_a if j < 64 else gi_b
                    jj = j % 64
                    po, oo = j * W2, (j + 1) * W2
                    gru_step(
                        wenc_sb,
                        lambda k, p0=po: h_ring[:, p0 + k * BS:p0 + (k + 1) * BS],
                        h_ring[:, po:po + W2].rearrange("p (k b) -> p k b", b=BS),
                        h_ring[:, oo:oo + W2].rearrange("p (k b) -> p k b", b=BS),
                        gi[:, 0:4, jj, :], gi[:, 4:6, jj, :], bhne_sb)
                    if j == 63:
                        nc.sync.dma_start(
                            out=h_all[:, bass.ds(iv * (2 * HB) + W2, HB)],
                            in_=h_ring[:, W2:W2 + HB])
                        emit_gi_block(load_xblk(iv * 1024 + 1024), gi_a)
                nc.sync.dma_start(
                    out=h_all[:, bass.ds(iv * (2 * HB) + W2 + HB, HB)],
                    in_=h_ring[:, W2 + HB:W2 + 2 * HB])
                emit_gi_block(load_xblk(iv * 1024 + 1536), gi_b)
                nc.vector.tensor_copy(h_ring[:, 0:W2],
                                      h_ring[:, BODY * W2:(BODY + 1) * W2])

            # ---- attention (constant across decoder steps) ----
            ha4 = h_all[:].rearrange("p (s k b) -> p s k b", k=2, b=BS)
            E4 = E_bc[:].rearrange("p (t b) -> p t b", b=BS)
            for nbk in range(2 * nb):
                ps = psbig.tile([128, 512], F32)
                for kh in (0, 1):
                    nc.tensor.matmul(
                        ps[:].rearrange("p (t b) -> p t b", b=BS),
                        lhsT=wah_sb[:, kh * 128:(kh + 1) * 128],
                        rhs=ha4[:, 1 + nbk * 64:1 + (nbk + 1) * 64, kh, :],
                        start=(kh == 0), stop=(kh == 1))
                nc.scalar.activation(E_bc[:, nbk * 512:(nbk + 1) * 512],
                                     ps[:], AF.Exp)
            S_bc = gates.tile([128, BS], F32)
            for b in range(BS):
                nc.vector.tensor_reduce(S_bc[:, b:b + 1], E4[:, :, b],
                                        axis=mybir.AxisListType.X, op=ALU.add)
            rinv = gates.tile([128, BS], F32)
            nc.vector.reciprocal(rinv[:], S_bc[:])
            for kh in (0, 1):
                c_raw = gates.tile([128, BS], F32)
                for b in range(BS):
                    nc.vector.tensor_mul(ttr_scr[:], ha4[:, 1:Ls + 1, kh, b],
                                         E4[:, :, b])
                    nc.vector.tensor_reduce(c_raw[:, b:b + 1], ttr_scr[:],
                                            axis=mybir.AxisListType.X,
                                            op=ALU.add)
                nc.vector.tensor_mul(c16[:, kh, :], c_raw[:], rinv[:])
            # gi_d = W_ih_d @ c + biases (constant for all decoder steps)
            psd = psg.tile([128, 6, BS], F32, tag="ps")
            for g in range(6):
                for k in (0, 1):
                    nc.tensor.matmul(
                        psd[:, g, :],
                        lhsT=wihd_sb[:, (k * 6 + g) * 128:(k * 6 + g + 1) * 128],
                        rhs=c16[:, k, :],
                        start=(k == 0), stop=(k == 1))
            nc.vector.tensor_add(gid_full[:], psd[:], gid_bias_sb[:])
            # gidT[b, j] = gid_rz[j, b] for j < 512, computed directly:
            # c.T @ W_ih_d.T via the same wihd tiles as moving operand,
            # bias added as a K=1 ones-row matmul.
            pgt = psbig.tile([128, 512], F32)
            for k in (0, 1):
                nc.tensor.matmul(
                    pgt[0:BS, :], lhsT=c16[:, k, :],
                    rhs=wihd_sb[:, k * 768:k * 768 + 512],
                    start=(k == 0), stop=False)
            ones1 = gates.tile([1, BS], F16, tag="ones1", name="ones1")
            nc.vector.memset(ones1[:], 1.0)
            nc.tensor.matmul(pgt[0:BS, :], lhsT=ones1[:], rhs=gidbrow_sb[:],
                             start=False, stop=True)
            nc.scalar.activation(gidT_sb[:], pgt[0:BS, :], AF.Identity)

            # ---- decoder scan ----
            nc.vector.tensor_copy(
                s_ring[:, 0:W2], h_all[:, Ls * W2:(Ls + 1) * W2])
            with tc.For_i(0, nb, 1, hint_engines=HE,
                          staggered_reset=True) as iv:
                for j in range(BODY):
                    po, oo = j * W2, (j + 1) * W2
                    gru_step(
                        wdec_sb,
                        lambda k, p0=po: s_ring[:, p0 + k * BS:p0 + (k + 1) * BS],
                        s_ring[:, po:po + W2].rearrange("p (k b) -> p k b", b=BS),
                        s_ring[:, oo:oo + W2].rearrange("p (k b) -> p k b", b=BS),
                        gid_full[:, 0:4, :], gid_full[:, 4:6, :], bhnd_sb,
                        gidT=gidT_sb)
                    if j == 63:
                        nc.sync.dma_start(
                            out=s_all[:, bass.ds(iv * (2 * HB), HB)],
                            in_=s_ring[:, W2:W2 + HB])
                nc.sync.dma_start(
                    out=s_all[:, bass.ds(iv * (2 * HB) + HB, HB)],
                    in_=s_ring[:, W2 + HB:W2 + 2 * HB])
                nc.vector.tensor_copy(s_ring[:, 0:W2],
                                      s_ring[:, BODY * W2:(BODY + 1) * W2])

            # ---- y head: y[i, b] = sigmoid(s_{i+1} . w_do + b_do) ----
            # y128[(di*8+b), cc] = y[cc*16+di, b]; s_all col i*16 holds s_{i+1}
            s4 = s_all[:].rearrange("p (s k b) -> p s k b", k=2, b=BS)
            for ccb in range(nb):
                pyt = psg.tile([128, BS], F32, tag="ps")
                for ccm in range(8):
                    cc = ccb * 8 + ccm
                    for kh in (0, 1):
                        # pack strided s-slice into contiguous lhsT
                        sp = gates.tile([128, 128], F16, tag="spack")
                        nc.vector.tensor_copy(
                            sp[:].rearrange("p (d b) -> p d b", b=BS),
                            s4[:, cc * 16:(cc + 1) * 16, kh, :])
                        nc.tensor.matmul(
                            pyt[:, ccm:ccm + 1],
                            lhsT=sp[:],
                            rhs=wdo_sb[:, kh:kh + 1],
                            start=(kh == 0), stop=(kh == 1))
                nc.scalar.activation(y128[:, ccb * 8:(ccb + 1) * 8], pyt[:],
                                     AF.Sigmoid, bias=bdo_sb[:])

            # ---- output head: out.T[o, b] = sum_i W_out[o, i] y[i, b] ----
            pso = psg.tile([128, BS], F32, tag="ps")
            NCC = nb * 8
            for cc in range(NCC):
                yx = gates.tile([128, BS], F16)
                nc.vector.tensor_scalar_mul(yx[:], bmask_sb[:],
                                            y128[:, cc:cc + 1])
                nc.tensor.matmul(
                    pso[:], lhsT=wout_sb[:, cc * 128:(cc + 1) * 128],
                    rhs=yx[:],
                    start=(cc == 0), stop=(cc == NCC - 1))
            nc.scalar.activation(out_sb[:], pso[:], AF.Identity,
                                 bias=bout_sb[:])
            nc.sync.dma_start(out=out_t[:], in_=out_sb[:])

    nc.compile()       # Bacc: register allocation + fusion passes
    return nc


def prep_inputs(x, W_ih_e, W_hh_e, b_ih_e, b_hh_e, W_ih_d, W_hh_d, b_ih_d,
                b_hh_d, W_dec_out, b_dec_out, W_attn, b_attn, W_out, b_out,
                nb=NB):
    """Host-side layout prep. Returns (shared_map, per_core_maps)."""
    f16 = np.float16
    Ls = nb * BODY

    def tiles_T(W):  # W [768, 256] -> lhsT tiles [(k*6+g)] as [128, 1536]
        Wt = W.T.astype(f16)  # [256, 768]
        cols = np.concatenate(
            [Wt[k * 128:(k + 1) * 128, g * 128:(g + 1) * 128]
             for k in range(2) for g in range(6)], axis=1)
        return np.ascontiguousarray(cols)

    shared = {
        "wenc": tiles_T(W_hh_e),
        "wdec": tiles_T(W_hh_d),
        "wihd": tiles_T(W_ih_d),
        "wihe": np.ascontiguousarray(W_ih_e.T.astype(f16)),          # [64, 768]
        "gibias_e": np.stack(
            [(b_ih_e + b_hh_e)[g * 128:(g + 1) * 128] if g < 4
             else b_ih_e[512 + (g - 4) * 128: 512 + (g - 3) * 128]
             for g in range(6)], axis=1).astype(np.float32),
        "gidbias": np.stack(
            [np.repeat(((b_ih_d + b_hh_d)[g * 128:(g + 1) * 128] if g < 4
                        else b_ih_d[512 + (g - 4) * 128: 512 + (g - 3) * 128]
                        )[:, None], BS, 1)
             for g in range(6)], axis=1).astype(np.float32),
        "bhhn_e": np.repeat(
            b_hh_e[512:].reshape(2, 128).T[:, :, None], BS, 2
        ).astype(np.float32),
        "bhhn_d": np.repeat(
            b_hh_d[512:].reshape(2, 128).T[:, :, None], BS, 2
        ).astype(np.float32),
        "wah_rep": np.concatenate(
            [np.repeat(W_attn[0, H + kh * 128: H + (kh + 1) * 128][:, None],
                       128, 1) for kh in range(2)], axis=1).astype(f16),
        "wdo1": W_dec_out[0].reshape(2, 128).T.astype(f16),
        "bdo": np.full((128, 1), float(np.asarray(b_dec_out).ravel()[0]),
                       np.float32),
        "bmask": np.tile(np.eye(BS, dtype=f16), (16, 1)),
        "ident8": np.eye(BS, dtype=f16),
        "gidbrow": ((b_ih_d + b_hh_d)[:512]).astype(f16).reshape(1, 512),
        # woutm[(di*8+b), cc*128+o] = W_out[o, cc*16+di]  (b-replicated)
        "wout": np.ascontiguousarray(
            np.repeat(
                W_out[:, :Ls].T.astype(f16).reshape(Ls // 16, 16, OUT),
                BS, axis=1
            ).reshape(Ls // 16, 128, OUT)        # [cc, (di b), o]
            .transpose(1, 0, 2).reshape(128, (Ls // 16) * OUT)),
        "bout": b_out.reshape(128, 1).astype(np.float32),
    }
    per_core = []
    xw = nb * 1024 + 1024
    for c in range(NCORES):
        xs = x[c * BS:(c + 1) * BS, :Ls]                  # [BS, Ls, P]
        xT = np.zeros((P, xw), f16)
        xT[:, :Ls * BS] = xs.transpose(2, 1, 0).reshape(P, Ls * BS)
        per_core.append({"xT": np.ascontiguousarray(xT), **shared})
    return per_core


_prog_cache = {}


def kernel(**inputs):
    inputs = {k: np.asarray(v) for k, v in inputs.items()}
    if "prog" not in _prog_cache:
        _prog_cache["prog"] = build_program(NB)
    nc = _prog_cache["prog"]
    in_maps = prep_inputs(**inputs, nb=NB)
    res = run_bass_kernel_spmd(nc, in_maps, core_ids=list(range(NCORES)))
    outs = []
    for c in range(NCORES):
        outs.append(res.results[c]["out_t"].T)            # [BS, 128]
    return np.concatenate(outs, axis=0).astype(np.float32)

